# revision 1
# baseline (speedup 1.0000x reference)
"""Concordance-index (C-index) kernel for Trainium2, 8 NeuronCores.

Math
----
Reference computes, over all pairs i<j of N=16384 samples:
    cc = ((y_i>=y_j & yh_i>=yh_j & st_j) | (y_i<=y_j & yh_i<=yh_j & st_i)) & triu
    tp = ((y_i<=y_j & st_i) | (y_i>=y_j & st_j)) & triu
    out = sum(cc) / sum(tp)

Key reduction: columns with st_j = 0 contribute nothing to either count
(A1(i,j) = [y_i>=y_j]*[yh_i>=yh_j]*st_j and A2(i,j) = [y_i>=y_j]*st_j both
vanish), so the pairwise sweep is N x ns over (all i) x (event j only):
    sum(cc) = S1 - ns,  S1 = sum_{i, j in E} [y_i>=y_j][yh_i>=yh_j]
    sum(tp) = S2 - ns,  S2 = sum_{i, j in E} [y_i>=y_j],   ns = |E|
(exact up to pairs simultaneously tied in y and yh — absent here).

Sharding: the ns event samples are packed into NCORES*JT_E*128 j-slots
(j on SBUF partitions, JT_E j-tiles per core); unused slots are padded
with y=yh=+BIG, which contributes exactly zero through every formula
below.  i is streamed along the free axis in F=4096 DMA-broadcast tiles.

Per (i-tile it, j-tile jt), col = it*JT_E+jt:
    g = sign(y_i - y_j)     ScalarE Sign + fused row-sum -> acc_sg[col]
    h = sign(yh_i - yh_j)   ScalarE Sign + fused row-sum -> acc_sh[col]
        or (most cols) h01 = [yh_i >= yh_j] on VectorE with fused row-sum
    p = g*h                 VectorE tensor_tensor (2x mode)
    sum of p                TensorE ones-weight matmuls -> PSUM accumulator
                            (acc_ps for sign-h cols, acc_p01 for 01-h cols)
Host reconstructs S1/S2 with exact integer algebra in float64:
    sign-h cells: G*H = (gh + g + h + 1)/4      (diag corr +3/4 per event)
    01-h  cells: G*H = (g*h01 + h01)/2          (diag corr +1/2 per event)
    S2 = (sum_all g + n_tiles*Mt)/2 + ns/2
and mirrors the reference's float32 division.
"""

import math
import os
import sys

import numpy as np

for _p in ("/opt/trn_rl_repo", "/root/.axon_site", "/root/.axon_site/_ro/trn_rl_repo"):
    if os.path.isdir(_p) and _p not in sys.path:
        sys.path.append(_p)

import concourse.bacc as bacc
import concourse.bass as bass
import concourse.mybir as mybir
from concourse import bass_utils
from concourse import tile

N = 16384
P = 128
NCORES = 8
F = 4096                 # i-tile width (free axis)
IT = N // F              # 4 i-tiles
BIG = np.float32(1e30)

FP32 = mybir.dt.float32
BF16 = mybir.dt.bfloat16
Alu = mybir.AluOpType
ActF = mybir.ActivationFunctionType


def _act_h_cols(nt):
    """Columns whose h runs on ScalarE as sign (engine balance)."""
    want = max(1, round(nt * 8 / 36))
    return frozenset([c for c in range(nt) if c % 3 == 0][:want])


def _pe_h_cols(nt):
    """01-h columns whose column-sum goes to TensorE (rest use the fused
    VectorE accumulator, which runs at 1x)."""
    rest = [c for c in range(nt) if c not in _act_h_cols(nt)]
    return frozenset(c for i, c in enumerate(rest) if i % 7 < 5)


def build_bass(jt_e):
    nt = IT * jt_e
    act_h = _act_h_cols(nt)
    pe_h = _pe_h_cols(nt)
    nc = bacc.Bacc(debug=False, num_devices=NCORES)

    y_full = nc.dram_tensor("y_full", [1, N], FP32, kind="ExternalInput")
    yh_full = nc.dram_tensor("yh_full", [1, N], FP32, kind="ExternalInput")
    y_sl = nc.dram_tensor("y_sl", [P, jt_e], FP32, kind="ExternalInput")
    yh_sl = nc.dram_tensor("yh_sl", [P, jt_e], FP32, kind="ExternalInput")
    o_sg = nc.dram_tensor("o_sg", [P, nt], FP32, kind="ExternalOutput")
    o_sh = nc.dram_tensor("o_sh", [P, nt], FP32, kind="ExternalOutput")
    o_ps = nc.dram_tensor("o_ps", [1, 512], FP32, kind="ExternalOutput")
    o_p01 = nc.dram_tensor("o_p01", [1, 512], FP32, kind="ExternalOutput")
    o_h01 = nc.dram_tensor("o_h01", [1, 512], FP32, kind="ExternalOutput")

    n_mm_s = len(act_h) * (F // 512)
    n_mm_01 = (nt - len(act_h)) * (F // 512)
    n_mm_h = len(pe_h) * (F // 512)

    with tile.TileContext(nc) as tc:
        with (
            tc.tile_pool(name="const", bufs=1) as cpool,
            tc.tile_pool(name="bcast", bufs=2) as bpool,
            tc.tile_pool(name="work", bufs=5) as wpool,
            tc.tile_pool(name="psum", bufs=1, space="PSUM") as ppool,
        ):
            y_j = cpool.tile([P, jt_e], FP32)
            nc.sync.dma_start(out=y_j[:, :], in_=y_sl[:, :])
            yh_j = cpool.tile([P, jt_e], FP32)
            nc.sync.dma_start(out=yh_j[:, :], in_=yh_sl[:, :])
            neg_y = cpool.tile([P, jt_e], FP32)
            nc.vector.tensor_scalar_mul(neg_y[:, :], y_j[:, :], -1.0)
            neg_yh = cpool.tile([P, jt_e], FP32)
            nc.vector.tensor_scalar_mul(neg_yh[:, :], yh_j[:, :], -1.0)

            ones_w = cpool.tile([P, 1], BF16)
            nc.vector.memset(ones_w[:, :], 1.0)

            acc_sg = cpool.tile([P, nt], FP32)
            acc_sh = cpool.tile([P, nt], FP32)
            nc.vector.memset(acc_sh[:, :], 0.0)
            acc_ps = ppool.tile([1, 512], FP32)
            acc_p01 = ppool.tile([1, 512], FP32)
            acc_h01 = ppool.tile([1, 512], FP32)
            seen = {"ps": 0, "p01": 0, "h01": 0}
            n_mm = {"ps": n_mm_s, "p01": n_mm_01, "h01": n_mm_h}

            def pe_reduce(key, acc, src):
                for ch in range(F // 512):
                    seen[key] += 1
                    nc.tensor.matmul(
                        acc[0:1, 0:512],
                        ones_w[:, :],
                        src[:, ch * 512:(ch + 1) * 512],
                        start=(seen[key] == 1),
                        stop=(seen[key] == n_mm[key]),
                    )

            for it in range(IT):
                yib = bpool.tile([P, F], FP32, tag="yib")
                nc.sync.dma_start(
                    out=yib[:, :],
                    in_=y_full[0:1, it * F:(it + 1) * F].to_broadcast((P, F)),
                )
                yhib = bpool.tile([P, F], FP32, tag="yhib")
                nc.sync.dma_start(
                    out=yhib[:, :],
                    in_=yh_full[0:1, it * F:(it + 1) * F].to_broadcast((P, F)),
                )
                for jt in range(jt_e):
                    col = it * jt_e + jt
                    g = wpool.tile([P, F], BF16, tag="g")
                    nc.scalar.activation(
                        out=g[:, :], in_=yib[:, :], func=ActF.Sign,
                        bias=neg_y[:, jt:jt + 1], scale=1.0,
                        accum_out=acc_sg[:, col:col + 1],
                    )
                    h = wpool.tile([P, F], BF16, tag="h")
                    if col in act_h:
                        nc.scalar.activation(
                            out=h[:, :], in_=yhib[:, :], func=ActF.Sign,
                            bias=neg_yh[:, jt:jt + 1], scale=1.0,
                            accum_out=acc_sh[:, col:col + 1],
                        )
                    elif col in pe_h:
                        # plain 2x compare; column-sum via TensorE
                        nc.vector.tensor_scalar(
                            out=h[:, :], in0=yhib[:, :],
                            scalar1=yh_j[:, jt:jt + 1], scalar2=None,
                            op0=Alu.is_ge,
                        )
                        pe_reduce("h01", acc_h01, h)
                    else:
                        # accum mode: out = in0 op0 s1; accum = sum(out) op1 s2
                        nc.vector.tensor_scalar(
                            out=h[:, :], in0=yhib[:, :],
                            scalar1=yh_j[:, jt:jt + 1], scalar2=0.0,
                            op0=Alu.is_ge, op1=Alu.add,
                            accum_out=acc_sh[:, col:col + 1],
                        )
                    p = wpool.tile([P, F], BF16, tag="p")
                    nc.vector.tensor_tensor(
                        out=p[:, :], in0=g[:, :], in1=h[:, :], op=Alu.mult)
                    pe_reduce("ps" if col in act_h else "p01",
                              acc_ps if col in act_h else acc_p01, p)

            nc.sync.dma_start(out=o_sg[:, :], in_=acc_sg[:, :])
            nc.sync.dma_start(out=o_sh[:, :], in_=acc_sh[:, :])
            for acc, o in ((acc_ps, o_ps), (acc_p01, o_p01), (acc_h01, o_h01)):
                stg = cpool.tile([1, 512], FP32, tag=f"stg_{o.name}")
                nc.vector.tensor_copy(out=stg[:, :], in_=acc[0:1, 0:512])
                nc.sync.dma_start(out=o[:, :], in_=stg[:, :])

    nc.compile()
    return nc


_NC_CACHE = {}


def _get_nc(jt_e):
    if jt_e not in _NC_CACHE:
        _NC_CACHE[jt_e] = build_bass(jt_e)
    return _NC_CACHE[jt_e]


def _shard(y, yh, status):
    """Pack event samples into j-slots; pad with +BIG (zero contribution)."""
    ev = np.nonzero(status == 1)[0]
    ns = len(ev)
    jt_e = max(1, math.ceil(ns / (NCORES * P)))
    slots = NCORES * jt_e * P
    y_e = np.full(slots, BIG, dtype=np.float32)
    yh_e = np.full(slots, BIG, dtype=np.float32)
    y_e[:ns] = y[ev]
    yh_e[:ns] = yh[ev]
    return ev, jt_e, y_e, yh_e


def make_in_maps(y, y_hat, status, shard):
    y = np.ascontiguousarray(np.asarray(y, dtype=np.float32))
    yh = np.ascontiguousarray(np.asarray(y_hat, dtype=np.float32))
    ev, jt_e, y_e, yh_e = shard
    y2 = y.reshape(1, N)
    yh2 = yh.reshape(1, N)
    per = jt_e * P
    in_maps = []
    for c in range(NCORES):
        sl = slice(c * per, (c + 1) * per)
        in_maps.append({
            "y_full": y2,
            "yh_full": yh2,
            # slot s = c*per + t*P + p  ->  [p, t]
            "y_sl": np.ascontiguousarray(y_e[sl].reshape(jt_e, P).T),
            "yh_sl": np.ascontiguousarray(yh_e[sl].reshape(jt_e, P).T),
        })
    return in_maps


def combine(results, status, shard):
    """Exact integer algebra (float64) over device partial sums."""
    ev, jt_e, y_e, yh_e = shard
    ns = float(len(ev))
    nt = IT * jt_e
    act_h = _act_h_cols(nt)
    Mt = float(P) * float(F)
    per = jt_e * P
    S1 = 0.0
    S2 = 0.0
    for c, r in enumerate(results):
        sg = r["o_sg"].astype(np.float64)
        sh = r["o_sh"].astype(np.float64)
        A_s = float(r["o_ps"].astype(np.float64).sum())
        A_01 = float(r["o_p01"].astype(np.float64).sum())
        s_cols = sorted(act_h)
        o_cols = [x for x in range(nt) if x not in act_h]
        B_s = float(sg[:, s_cols].sum())
        C_s = float(sh[:, s_cols].sum())
        # 01-column h sums: PE accumulator for pe_h cols, fused DVE
        # accumulator (o_sh columns) for the rest
        C_01 = float(r["o_h01"].astype(np.float64).sum())
        C_01 += float(sh[:, [x for x in o_cols if x not in _pe_h_cols(nt)]].sum())
        S1 += (A_s + B_s + C_s + len(s_cols) * Mt) / 4.0
        S1 += (A_01 + C_01) / 2.0
        S2 += (float(sg.sum()) + nt * Mt) / 2.0
    # diagonal corrections: event e in slot s pairs with itself at
    # i-tile it_e = ev[s]//F, j-tile jt = (s % per)//P of core s//per.
    for s, orig in enumerate(ev):
        jt_e_local = (s % per) // P
        col = (orig // F) * jt_e + jt_e_local
        S1 += 0.75 if col in act_h else 0.5
    S2 += ns / 2.0
    c32 = np.float32(S1 - ns)
    t32 = np.float32(S2 - ns)
    return np.asarray(np.float32(c32 / t32))


def kernel(y, y_hat, status, _run_kwargs=None):
    status = np.asarray(status)
    shard = _shard(np.asarray(y), np.asarray(y_hat), status)
    nc = _get_nc(shard[1])
    in_maps = make_in_maps(y, y_hat, status, shard)
    kw = dict(_run_kwargs or {})
    res = bass_utils.run_bass_kernel_spmd(
        nc, in_maps, core_ids=list(range(NCORES)), **kw)
    out = combine(res.results, status, shard)
    if _run_kwargs is not None:
        return out, res
    return out


if __name__ == "__main__":
    rng = np.random.default_rng(0)
    y = rng.standard_normal(N).astype(np.float32)
    yh = rng.standard_normal(N).astype(np.float32)
    st = (rng.integers(0, 2, N)).astype(np.int32)
    print(kernel(y, yh, st))



# revision 3
# speedup vs baseline: 1.2736x; 1.2736x over previous
"""Concordance-index (C-index) kernel for Trainium2, 8 NeuronCores — v2.

Math
----
Reference computes, over all pairs i<j of N=16384 samples:
    cc = ((y_i>=y_j & yh_i>=yh_j & st_j) | (y_i<=y_j & yh_i<=yh_j & st_i)) & triu
    tp = ((y_i<=y_j & st_i) | (y_i>=y_j & st_j)) & triu
    out = sum(cc) / sum(tp)

Columns with st_j = 0 contribute nothing, so the sweep is N x ns over
(all i) x (event j): with A = [y_i >= y_j], B = [yh_i >= yh_j],
    sum(cc) = S1 - ns,  S1 = sum_{i, j in E} A*B   (diag = 1 each)
    sum(tp) = S2 - ns,  S2 = sum_{i, j in E} A
(exact up to double-tied pairs — negligible).

v2 design (HW-measured op menu, all-bf16 tiles):
  A-cols ("01"): DVE ts-plain a01 (fast 3.5x mode, 1285ns/tile) +
      DVE stt fused (yh-compare * a01, row-accum -> S1 part, 4485ns) +
      PE ones-matmul counts a01 -> S2 part (parallel engine).
  B-cols ("sign"): ScalarE Sign g,h with fused accums (3707ns each) +
      DVE tt product (2x, 2291ns) + PE ones-matmul sum -> sign algebra
      (M + g + h + gh)/4 and (M + g)/2.
This balances DVE ~ ACT ~ PE ~ 130us, vs 186us for v1 (which burned
DVE+ACT on 3 elementwise ops/cell and PE on all reductions).

Inputs are pre-rounded to bf16 on host (y_sl values = fp32(bf16(y)))
so on-chip compares are consistent; bf16 ties add ~0.1% noise, well
inside the 2e-2 gate. Host reconstructs S1/S2 in float64 and mirrors
the reference's float32 division.
"""

import math
import os
import sys

import numpy as np

for _p in ("/opt/trn_rl_repo", "/root/.axon_site", "/root/.axon_site/_ro/trn_rl_repo"):
    if os.path.isdir(_p) and _p not in sys.path:
        sys.path.append(_p)

import ml_dtypes

import concourse.bacc as bacc
import concourse.mybir as mybir
from concourse import bass_utils
from concourse import tile

N = 16384
P = 128
NCORES = 8
F = 4096                 # i-tile width (free axis)
IT = N // F              # 4 i-tiles
BIG = np.float32(1e30)

FP32 = mybir.dt.float32
BF16 = mybir.dt.bfloat16
Alu = mybir.AluOpType
ActF = mybir.ActivationFunctionType


def _a_cols(nt):
    """Columns using the 01 flavor (DVE ts+stt, PE counts a01)."""
    return frozenset(c for c in range(nt) if c % 2 == 0)


def build_bass(jt_e):
    nt = IT * jt_e
    acols = _a_cols(nt)
    nc = bacc.Bacc(debug=False, num_devices=NCORES)

    y_full = nc.dram_tensor("y_full", [1, N], BF16, kind="ExternalInput")
    yh_full = nc.dram_tensor("yh_full", [1, N], BF16, kind="ExternalInput")
    y_sl = nc.dram_tensor("y_sl", [P, jt_e], FP32, kind="ExternalInput")
    yh_sl = nc.dram_tensor("yh_sl", [P, jt_e], FP32, kind="ExternalInput")
    o_r = nc.dram_tensor("o_r", [P, nt], FP32, kind="ExternalOutput")
    o_g = nc.dram_tensor("o_g", [P, nt], FP32, kind="ExternalOutput")
    o_h = nc.dram_tensor("o_h", [P, nt], FP32, kind="ExternalOutput")
    o_pa = nc.dram_tensor("o_pa", [1, 512], FP32, kind="ExternalOutput")
    o_pp = nc.dram_tensor("o_pp", [1, 512], FP32, kind="ExternalOutput")

    n_mm = {"pa": len(acols) * (F // 512),
            "pp": (nt - len(acols)) * (F // 512)}

    with tile.TileContext(nc) as tc:
        with (
            tc.tile_pool(name="const", bufs=1) as cpool,
            tc.tile_pool(name="bcast", bufs=2) as bpool,
            tc.tile_pool(name="work", bufs=3) as wpool,
            tc.tile_pool(name="psum", bufs=1, space="PSUM") as ppool,
        ):
            y_j = cpool.tile([P, jt_e], FP32)
            nc.sync.dma_start(out=y_j[:, :], in_=y_sl[:, :])
            yh_j = cpool.tile([P, jt_e], FP32)
            nc.sync.dma_start(out=yh_j[:, :], in_=yh_sl[:, :])
            neg_y = cpool.tile([P, jt_e], FP32)
            nc.vector.tensor_scalar_mul(neg_y[:, :], y_j[:, :], -1.0)
            neg_yh = cpool.tile([P, jt_e], FP32)
            nc.vector.tensor_scalar_mul(neg_yh[:, :], yh_j[:, :], -1.0)

            ones_w = cpool.tile([P, 1], BF16)
            nc.vector.memset(ones_w[:, :], 1.0)

            acc_r = cpool.tile([P, nt], FP32)
            nc.vector.memset(acc_r[:, :], 0.0)
            acc_g = cpool.tile([P, nt], FP32)
            nc.vector.memset(acc_g[:, :], 0.0)
            acc_h = cpool.tile([P, nt], FP32)
            nc.vector.memset(acc_h[:, :], 0.0)
            acc_pa = ppool.tile([1, 512], FP32)
            acc_pp = ppool.tile([1, 512], FP32)
            seen = {"pa": 0, "pp": 0}

            def pe_reduce(key, acc, src):
                for ch in range(F // 512):
                    seen[key] += 1
                    nc.tensor.matmul(
                        acc[0:1, 0:512],
                        ones_w[:, :],
                        src[:, ch * 512:(ch + 1) * 512],
                        start=(seen[key] == 1),
                        stop=(seen[key] == n_mm[key]),
                    )

            for it in range(IT):
                yib = bpool.tile([P, F], BF16, tag="yib")
                nc.sync.dma_start(
                    out=yib[:, :],
                    in_=y_full[0:1, it * F:(it + 1) * F].to_broadcast((P, F)),
                )
                yhib = bpool.tile([P, F], BF16, tag="yhib")
                nc.sync.dma_start(
                    out=yhib[:, :],
                    in_=yh_full[0:1, it * F:(it + 1) * F].to_broadcast((P, F)),
                )
                for jt in range(jt_e):
                    col = it * jt_e + jt
                    if col in acols:
                        a01 = wpool.tile([P, F], BF16, tag="a01")
                        nc.vector.tensor_scalar(
                            out=a01[:, :], in0=yib[:, :],
                            scalar1=y_j[:, jt:jt + 1], scalar2=None,
                            op0=Alu.is_ge,
                        )
                        pab = wpool.tile([P, F], BF16, tag="pab")
                        nc.vector.scalar_tensor_tensor(
                            out=pab[:, :], in0=yhib[:, :],
                            scalar=yh_j[:, jt:jt + 1], in1=a01[:, :],
                            op0=Alu.is_ge, op1=Alu.mult,
                            accum_out=acc_r[:, col:col + 1],
                        )
                        pe_reduce("pa", acc_pa, a01)
                    else:
                        g = wpool.tile([P, F], BF16, tag="g")
                        nc.scalar.activation(
                            out=g[:, :], in_=yib[:, :], func=ActF.Sign,
                            bias=neg_y[:, jt:jt + 1], scale=1.0,
                            accum_out=acc_g[:, col:col + 1],
                        )
                        h = wpool.tile([P, F], BF16, tag="h")
                        nc.scalar.activation(
                            out=h[:, :], in_=yhib[:, :], func=ActF.Sign,
                            bias=neg_yh[:, jt:jt + 1], scale=1.0,
                            accum_out=acc_h[:, col:col + 1],
                        )
                        p = wpool.tile([P, F], BF16, tag="p")
                        nc.vector.tensor_tensor(
                            out=p[:, :], in0=g[:, :], in1=h[:, :], op=Alu.mult)
                        pe_reduce("pp", acc_pp, p)

            nc.sync.dma_start(out=o_r[:, :], in_=acc_r[:, :])
            nc.sync.dma_start(out=o_g[:, :], in_=acc_g[:, :])
            nc.sync.dma_start(out=o_h[:, :], in_=acc_h[:, :])
            for acc, o in ((acc_pa, o_pa), (acc_pp, o_pp)):
                stg = cpool.tile([1, 512], FP32, tag=f"stg_{o.name}")
                nc.vector.tensor_copy(out=stg[:, :], in_=acc[0:1, 0:512])
                nc.sync.dma_start(out=o[:, :], in_=stg[:, :])

    nc.compile()
    return nc


_NC_CACHE = {}


def _get_nc(jt_e):
    if jt_e not in _NC_CACHE:
        _NC_CACHE[jt_e] = build_bass(jt_e)
    return _NC_CACHE[jt_e]


def _shard(y, yh, status):
    """Pack event samples into j-slots; pad with +BIG (zero contribution).

    Values are pre-rounded to bf16 so scalar compares match the bf16
    broadcast tiles exactly.
    """
    ev = np.nonzero(status == 1)[0]
    ns = len(ev)
    jt_e = max(1, math.ceil(ns / (NCORES * P)))
    slots = NCORES * jt_e * P
    y_e = np.full(slots, BIG, dtype=np.float32)
    yh_e = np.full(slots, BIG, dtype=np.float32)
    y_e[:ns] = y[ev].astype(ml_dtypes.bfloat16).astype(np.float32)
    yh_e[:ns] = yh[ev].astype(ml_dtypes.bfloat16).astype(np.float32)
    return ev, jt_e, y_e, yh_e


def make_in_maps(y, y_hat, status, shard):
    y = np.asarray(y, dtype=np.float32)
    yh = np.asarray(y_hat, dtype=np.float32)
    ev, jt_e, y_e, yh_e = shard
    y2 = np.ascontiguousarray(y.astype(ml_dtypes.bfloat16).reshape(1, N))
    yh2 = np.ascontiguousarray(yh.astype(ml_dtypes.bfloat16).reshape(1, N))
    per = jt_e * P
    in_maps = []
    for c in range(NCORES):
        sl = slice(c * per, (c + 1) * per)
        in_maps.append({
            "y_full": y2,
            "yh_full": yh2,
            # slot s = c*per + t*P + p  ->  [p, t]
            "y_sl": np.ascontiguousarray(y_e[sl].reshape(jt_e, P).T),
            "yh_sl": np.ascontiguousarray(yh_e[sl].reshape(jt_e, P).T),
        })
    return in_maps


def combine(results, status, shard):
    """Reconstruct S1/S2 from device partial sums (float64 algebra)."""
    ev, jt_e, y_e, yh_e = shard
    ns = float(len(ev))
    nt = IT * jt_e
    acols = _a_cols(nt)
    bcols = [c for c in range(nt) if c not in acols]
    Mt = float(P) * float(F)
    per = jt_e * P
    S1 = 0.0
    S2 = 0.0
    for c, r in enumerate(results):
        rr = r["o_r"].astype(np.float64)
        gg = r["o_g"].astype(np.float64)
        hh = r["o_h"].astype(np.float64)
        pa = float(r["o_pa"].astype(np.float64).sum())
        pp = float(r["o_pp"].astype(np.float64).sum())
        Sg = float(gg[:, bcols].sum())
        Sh = float(hh[:, bcols].sum())
        S1 += float(rr[:, sorted(acols)].sum())
        S1 += (pp + Sg + Sh + len(bcols) * Mt) / 4.0
        S2 += pa
        S2 += (Sg + len(bcols) * Mt) / 2.0
    # diagonal corrections for B-cols (sign algebra puts 1/4 resp. 1/2
    # on the g=h=0 diag cell; rect semantics want 1)
    for s, orig in enumerate(ev):
        jt_loc = (s % per) // P
        col = (orig // F) * jt_e + jt_loc
        if col not in acols:
            S1 += 0.75
            S2 += 0.5
    c32 = np.float32(S1 - ns)
    t32 = np.float32(S2 - ns)
    return np.asarray(np.float32(c32 / t32))


def kernel(y, y_hat, status, _run_kwargs=None):
    status = np.asarray(status)
    shard = _shard(np.asarray(y), np.asarray(y_hat), status)
    nc = _get_nc(shard[1])
    in_maps = make_in_maps(y, y_hat, status, shard)
    kw = dict(_run_kwargs or {})
    res = bass_utils.run_bass_kernel_spmd(
        nc, in_maps, core_ids=list(range(NCORES)), **kw)
    out = combine(res.results, status, shard)
    if _run_kwargs is not None:
        return out, res
    return out


if __name__ == "__main__":
    rng = np.random.default_rng(0)
    y = rng.standard_normal(N).astype(np.float32)
    yh = rng.standard_normal(N).astype(np.float32)
    st = (rng.integers(0, 2, N)).astype(np.int32)
    print(kernel(y, yh, st))


# revision 9
# speedup vs baseline: 2.2292x; 1.7503x over previous
"""Concordance-index (C-index) kernel for Trainium2, 8 NeuronCores — v3.

Math
----
Reference computes, over all pairs i<j of N=16384 samples:
    cc = ((y_i>=y_j & yh_i>=yh_j & st_j) | (y_i<=y_j & yh_i<=yh_j & st_i)) & triu
    tp = ((y_i<=y_j & st_i) | (y_i>=y_j & st_j)) & triu
    out = sum(cc) / sum(tp)

Columns with st_j = 0 contribute nothing, so the sweep is N x ns over
(all i) x (event j): with A = [y_i >= y_j], B = [yh_i >= yh_j],
    sum(cc) = S1 - ns,  S1 = sum_{i, j in E} A*B   (diag = 1 each)
    sum(tp) = S2 - ns,  S2 = sum_{i, j in E} A

v3 key idea: the host permutes the i-axis to y-sorted order and packs
event columns y-sorted + rank-interleaved across cores.  Then per
128-event j-group, A is a step function along i, and most [128, 4096]
i-tiles are fully decided:
    "zero" tile (all y_i < all y_j): contributes nothing -> SKIPPED
    "ones" tile (all y_i >= all y_j): A==1, so only sum(B) is needed ->
        one DVE ts-accum (or ScalarE sign-accum) per tile
    "mixed" tile (~1.3 per group): full compute, flavored A (DVE
        ts-plain a01 + stt fused b*a with row-accum; PE counts a01) or
        B (2 ScalarE Signs + DVE tt product + PE ones-matmul) to
        balance DVE/ACT/PE.
This cuts tile-ops from 32 full to ~11 mixed + ~9 cheap per core.
The (pattern, flavors) are data-dependent -> compiled per pattern and
cached. Pads use y_j=-BIG / yh_j=+BIG so every formula contributes an
exact host-known constant. bf16 tie noise ~5e-4 total, gate is 2e-2.
"""

import math
import os
import sys

import numpy as np

for _p in ("/opt/trn_rl_repo", "/root/.axon_site", "/root/.axon_site/_ro/trn_rl_repo"):
    if os.path.isdir(_p) and _p not in sys.path:
        sys.path.append(_p)

import ml_dtypes

import concourse.bacc as bacc
import concourse.mybir as mybir
from concourse import bass_utils
from concourse import tile

N = 16384
P = 128
NCORES = 8
F = 4096                 # i-tile width (free axis)
IT = N // F              # 4 i-tiles
BIG = np.float32(1e30)

FP32 = mybir.dt.float32
BF16 = mybir.dt.bfloat16
Alu = mybir.AluOpType
ActF = mybir.ActivationFunctionType

# tile classes
ZERO, ONES_DVE, ONES_ACT, MIX_A, MIX_B = 0, 1, 2, 3, 4


def _bf(x):
    return np.asarray(x, np.float32).astype(ml_dtypes.bfloat16).astype(np.float32)


class Shard:
    """Host-side packing: y-sorted i-axis, rank-interleaved event slots,
    per-(group, itile) class pattern."""

    def __init__(self, y, yh, status):
        y = np.asarray(y, np.float32)
        yh = np.asarray(yh, np.float32)
        ybf = _bf(y)
        yhbf = _bf(yh)
        # i-axis permutation: stable sort by bf16 y
        self.idx = np.argsort(ybf, kind="stable")
        self.y_sorted = ybf[self.idx]          # fed as y_full
        self.yh_perm = yhbf[self.idx]          # fed as yh_full
        pos_of_orig = np.empty(N, np.int64)
        pos_of_orig[self.idx] = np.arange(N)

        ev = np.nonzero(np.asarray(status) == 1)[0]
        self.ns = len(ev)
        # events sorted by bf16 y, rank r -> core r % NCORES
        ev_sorted = ev[np.argsort(ybf[ev], kind="stable")]
        self.jt_e = max(1, math.ceil(self.ns / (NCORES * P)))
        per = self.jt_e * P
        self.per = per
        self.nt = IT * self.jt_e

        # slot tables per core: orig index (-1 = pad), y/yh scalar values
        self.slot_orig = np.full((NCORES, per), -1, np.int64)
        self.y_sl = np.full((NCORES, per), -BIG, np.float32)
        self.yh_sl = np.full((NCORES, per), BIG, np.float32)
        for c in range(NCORES):
            mine = ev_sorted[c::NCORES]
            k = len(mine)
            self.slot_orig[c, :k] = mine
            self.y_sl[c, :k] = ybf[mine]
            self.yh_sl[c, :k] = yhbf[mine]

        # crossing positions: first sorted-i with y_i >= y_j
        # (exact: both sides bf16 values)
        ssl = np.searchsorted(self.y_sorted, self.y_sl.reshape(-1),
                              side="left").reshape(NCORES, per)

        # per-(group, itile) class, unioned across cores
        cls = np.zeros((self.jt_e, IT), np.int64)
        self.group_lo = np.zeros((NCORES, self.jt_e), np.int64)
        self.group_hi = np.zeros((NCORES, self.jt_e), np.int64)
        for g in range(self.jt_e):
            lo_all, hi_all = N, 0
            for c in range(NCORES):
                sl = slice(g * P, (g + 1) * P)
                real = self.slot_orig[c, sl] >= 0
                if real.any():
                    lo = int(ssl[c, sl][real].min())
                    hi = int(ssl[c, sl][real].max())
                else:
                    lo, hi = N, 0   # all-pad group: everything "zero"
                self.group_lo[c, g] = lo
                self.group_hi[c, g] = hi
                lo_all = min(lo_all, lo)
                hi_all = max(hi_all, hi)
            for it in range(IT):
                t0, t1 = it * F, (it + 1) * F
                if t1 <= lo_all:
                    cls[g, it] = ZERO
                elif t0 >= hi_all:
                    cls[g, it] = ONES_DVE
                else:
                    cls[g, it] = MIX_A
        self.cls = cls
        self._assign_flavors()
        # event diag position (sorted-i space) per core/slot
        self.pos_of_orig = pos_of_orig
        self.pattern = tuple(cls.reshape(-1).tolist())

    def _assign_flavors(self):
        """Balance DVE / ACT / PE by greedy assignment (measured ns/tile)."""
        cost = {MIX_A: (5639, 0, 4536), MIX_B: (2287, 7940, 4536),
                ONES_DVE: (4476, 0, 0), ONES_ACT: (0, 3990, 0)}
        load = [0.0, 0.0, 0.0]
        order = [(g, it) for g in range(self.jt_e) for it in range(IT)]
        # mixed first (larger), then ones
        for kind, opts in ((MIX_A, (MIX_A, MIX_B)),
                           (ONES_DVE, (ONES_DVE, ONES_ACT))):
            for g, it in order:
                if self.cls[g, it] != kind:
                    continue
                best, bestmax = None, None
                for o in opts:
                    trial = [load[k] + cost[o][k] for k in range(3)]
                    m = max(trial)
                    if bestmax is None or m < bestmax:
                        bestmax, best = m, o
                self.cls[g, it] = best
                for k in range(3):
                    load[k] += cost[best][k]
        self.load = load


def build_bass(shard):
    jt_e, nt, cls = shard.jt_e, shard.nt, shard.cls
    nc = bacc.Bacc(debug=False, num_devices=NCORES)

    y_full = nc.dram_tensor("y_full", [1, N], BF16, kind="ExternalInput")
    yh_full = nc.dram_tensor("yh_full", [1, N], BF16, kind="ExternalInput")
    y_sl = nc.dram_tensor("y_sl", [P, jt_e], FP32, kind="ExternalInput")
    yh_sl = nc.dram_tensor("yh_sl", [P, jt_e], FP32, kind="ExternalInput")
    o_r = nc.dram_tensor("o_r", [P, nt], FP32, kind="ExternalOutput")
    o_g = nc.dram_tensor("o_g", [P, nt], FP32, kind="ExternalOutput")
    o_h = nc.dram_tensor("o_h", [P, nt], FP32, kind="ExternalOutput")
    o_b = nc.dram_tensor("o_b", [P, nt], FP32, kind="ExternalOutput")
    o_pa = nc.dram_tensor("o_pa", [1, 512], FP32, kind="ExternalOutput")
    o_pp = nc.dram_tensor("o_pp", [1, 512], FP32, kind="ExternalOutput")

    n_mm = {"pa": int((cls == MIX_A).sum()) * (F // 512),
            "pp": int((cls == MIX_B).sum()) * (F // 512)}
    # which i-tiles are needed at all
    it_used = [it for it in range(IT)
               if any(cls[g, it] != ZERO for g in range(jt_e))]

    with tile.TileContext(nc) as tc:
        with (
            tc.tile_pool(name="const", bufs=1) as cpool,
            tc.tile_pool(name="bcast", bufs=2) as bpool,
            tc.tile_pool(name="work", bufs=3) as wpool,
            tc.tile_pool(name="psum", bufs=1, space="PSUM") as ppool,
        ):
            y_j = cpool.tile([P, jt_e], FP32)
            nc.sync.dma_start(out=y_j[:, :], in_=y_sl[:, :])
            yh_j = cpool.tile([P, jt_e], FP32)
            nc.sync.dma_start(out=yh_j[:, :], in_=yh_sl[:, :])
            neg_y = cpool.tile([P, jt_e], FP32)
            nc.vector.tensor_scalar_mul(neg_y[:, :], y_j[:, :], -1.0)
            neg_yh = cpool.tile([P, jt_e], FP32)
            nc.vector.tensor_scalar_mul(neg_yh[:, :], yh_j[:, :], -1.0)

            ones_w = cpool.tile([P, 1], BF16)
            nc.vector.memset(ones_w[:, :], 1.0)
            ones_t = cpool.tile([P, F], BF16)
            nc.vector.memset(ones_t[:, :], 1.0)

            accs = {}
            for nm in ("r", "g", "h", "b"):
                # no memset: combine() only reads columns their class's
                # accum op writes (accum_out overwrites, not adds)
                t = cpool.tile([P, nt], FP32, tag=f"acc_{nm}")
                accs[nm] = t
            acc_pa = ppool.tile([1, 512], FP32)
            acc_pp = ppool.tile([1, 512], FP32)
            seen = {"pa": 0, "pp": 0}

            def pe_reduce(key, acc, src):
                for ch in range(F // 512):
                    seen[key] += 1
                    nc.tensor.matmul(
                        acc[0:1, 0:512],
                        ones_w[:, :],
                        src[:, ch * 512:(ch + 1) * 512],
                        start=(seen[key] == 1),
                        stop=(seen[key] == n_mm[key]),
                    )

            for it in it_used:
                need_y = any(cls[g, it] in (MIX_A, MIX_B) for g in range(jt_e))
                yib = None
                if need_y:
                    yib = bpool.tile([P, F], BF16, tag="yib")
                    nc.sync.dma_start(
                        out=yib[:, :],
                        in_=y_full[0:1, it * F:(it + 1) * F].to_broadcast((P, F)),
                    )
                yhib = bpool.tile([P, F], BF16, tag="yhib")
                nc.sync.dma_start(
                    out=yhib[:, :],
                    in_=yh_full[0:1, it * F:(it + 1) * F].to_broadcast((P, F)),
                )
                for g in range(jt_e):
                    col = it * jt_e + g
                    k = cls[g, it]
                    if k == ZERO:
                        continue
                    if k == ONES_DVE:
                        # stt against a ones tile == is_ge with accum; avoids
                        # mixing ts-accum with stt-accum on DVE (scheduler
                        # deadlock) and keeps one accumulator op type
                        b01 = wpool.tile([P, F], BF16, tag="stt_out")
                        nc.vector.scalar_tensor_tensor(
                            out=b01[:, :], in0=yhib[:, :],
                            scalar=yh_j[:, g:g + 1], in1=ones_t[:, :],
                            op0=Alu.is_ge, op1=Alu.mult,
                            accum_out=accs["b"][:, col:col + 1],
                        )
                    elif k == ONES_ACT:
                        hs = wpool.tile([P, F], BF16, tag="hs")
                        nc.scalar.activation(
                            out=hs[:, :], in_=yhib[:, :], func=ActF.Sign,
                            bias=neg_yh[:, g:g + 1], scale=1.0,
                            accum_out=accs["h"][:, col:col + 1],
                        )
                    elif k == MIX_A:
                        a01 = wpool.tile([P, F], BF16, tag="a01")
                        nc.vector.tensor_scalar(
                            out=a01[:, :], in0=yib[:, :],
                            scalar1=y_j[:, g:g + 1], scalar2=None,
                            op0=Alu.is_ge,
                        )
                        pab = wpool.tile([P, F], BF16, tag="stt_out")
                        nc.vector.scalar_tensor_tensor(
                            out=pab[:, :], in0=yhib[:, :],
                            scalar=yh_j[:, g:g + 1], in1=a01[:, :],
                            op0=Alu.is_ge, op1=Alu.mult,
                            accum_out=accs["r"][:, col:col + 1],
                        )
                        pe_reduce("pa", acc_pa, a01)
                    else:  # MIX_B
                        gs = wpool.tile([P, F], BF16, tag="gs")
                        nc.scalar.activation(
                            out=gs[:, :], in_=yib[:, :], func=ActF.Sign,
                            bias=neg_y[:, g:g + 1], scale=1.0,
                            accum_out=accs["g"][:, col:col + 1],
                        )
                        hs = wpool.tile([P, F], BF16, tag="hs")
                        nc.scalar.activation(
                            out=hs[:, :], in_=yhib[:, :], func=ActF.Sign,
                            bias=neg_yh[:, g:g + 1], scale=1.0,
                            accum_out=accs["h"][:, col:col + 1],
                        )
                        p = wpool.tile([P, F], BF16, tag="p")
                        nc.vector.tensor_tensor(
                            out=p[:, :], in0=gs[:, :], in1=hs[:, :],
                            op=Alu.mult)
                        pe_reduce("pp", acc_pp, p)

            nc.sync.dma_start(out=o_r[:, :], in_=accs["r"][:, :])
            nc.sync.dma_start(out=o_g[:, :], in_=accs["g"][:, :])
            nc.sync.dma_start(out=o_h[:, :], in_=accs["h"][:, :])
            nc.sync.dma_start(out=o_b[:, :], in_=accs["b"][:, :])
            for acc, o, key in ((acc_pa, o_pa, "pa"), (acc_pp, o_pp, "pp")):
                stg = cpool.tile([1, 512], FP32, tag=f"stg_{o.name}")
                if n_mm[key] == 0:
                    nc.vector.memset(stg[:, :], 0.0)
                else:
                    nc.vector.tensor_copy(out=stg[:, :], in_=acc[0:1, 0:512])
                nc.sync.dma_start(out=o[:, :], in_=stg[:, :])

    nc.compile()
    return nc


_NC_CACHE = {}


def _get_nc(shard):
    key = (shard.jt_e, shard.pattern)
    if key not in _NC_CACHE:
        _NC_CACHE[key] = build_bass(shard)
    return _NC_CACHE[key]


def make_in_maps(shard):
    y2 = np.ascontiguousarray(
        shard.y_sorted.astype(ml_dtypes.bfloat16).reshape(1, N))
    yh2 = np.ascontiguousarray(
        shard.yh_perm.astype(ml_dtypes.bfloat16).reshape(1, N))
    in_maps = []
    for c in range(NCORES):
        in_maps.append({
            "y_full": y2,
            "yh_full": yh2,
            # slot s = g*P + p  ->  [p, g]
            "y_sl": np.ascontiguousarray(
                shard.y_sl[c].reshape(shard.jt_e, P).T),
            "yh_sl": np.ascontiguousarray(
                shard.yh_sl[c].reshape(shard.jt_e, P).T),
        })
    return in_maps


def combine(results, shard):
    """Exact reconstruction in float64 from device partial sums."""
    jt_e, nt, cls = shard.jt_e, shard.nt, shard.cls
    Ff = float(F)
    S1 = 0.0
    S2 = 0.0
    n_pad = (shard.slot_orig < 0).sum(axis=1)  # per core (in last group)
    for c, r in enumerate(results):
        rr = r["o_r"].astype(np.float64)
        gg = r["o_g"].astype(np.float64)
        hh = r["o_h"].astype(np.float64)
        bb = r["o_b"].astype(np.float64)
        pa = float(r["o_pa"].astype(np.float64).sum())
        pp = float(r["o_pp"].astype(np.float64).sum())
        real = (shard.slot_orig[c] >= 0).reshape(jt_e, P)  # [g, p]
        S2 += pa  # pad pollution removed below
        for g in range(jt_e):
            nreal = int(real[g].sum())
            npad = P - nreal
            for it in range(IT):
                col = it * jt_e + g
                k = cls[g, it]
                if k == ZERO:
                    continue
                if k == ONES_DVE:
                    # A==1 for real slots: S1 += sum(B), S2 += F per real
                    S1 += float(bb[:, col][real[g]].sum())
                    S2 += Ff * nreal
                elif k == ONES_ACT:
                    # sum(B) ~= (F + sum(h_sign))/2 per real partition
                    S1 += float(
                        (Ff + hh[:, col][real[g]]).sum()) / 2.0
                    S2 += Ff * nreal
                elif k == MIX_A:
                    S1 += float(rr[:, col].sum())   # pads contribute 0
                    S2 -= Ff * npad                  # pad a01==1 rows in pa
                else:  # MIX_B
                    # per-cell (1+g)(1+h)/4 identity: pads cancel exactly
                    S1 += (Ff * P + float(gg[:, col].sum())
                           + float(hh[:, col].sum())) / 4.0
                    # (F + sum g)/2 per partition; pad rows give F -> remove
                    S2 += (Ff * P + float(gg[:, col].sum())) / 2.0 - Ff * npad
        # gh product term of the MIX_B tiles (PSUM-accumulated per core)
        S1 += pp / 4.0
    # diagonal corrections for MIX_B tiles (est 1/4 resp 1/2, want 1)
    for c in range(NCORES):
        for s in range(shard.per):
            o = shard.slot_orig[c, s]
            if o < 0:
                continue
            g = s // P
            it = int(shard.pos_of_orig[o]) // F
            if cls[g, it] == MIX_B:
                S1 += 0.75
                S2 += 0.5
    ns = float(shard.ns)
    c32 = np.float32(S1 - ns)
    t32 = np.float32(S2 - ns)
    return np.asarray(np.float32(c32 / t32))


def kernel(y, y_hat, status, _run_kwargs=None):
    shard = Shard(y, y_hat, status)
    nc = _get_nc(shard)
    in_maps = make_in_maps(shard)
    kw = dict(_run_kwargs or {})
    res = bass_utils.run_bass_kernel_spmd(
        nc, in_maps, core_ids=list(range(NCORES)), **kw)
    out = combine(res.results, shard)
    if _run_kwargs is not None:
        return out, res
    return out


if __name__ == "__main__":
    rng = np.random.default_rng(0)
    y = rng.standard_normal(N).astype(np.float32)
    yh = rng.standard_normal(N).astype(np.float32)
    st = (rng.integers(0, 2, N)).astype(np.int32)
    print(kernel(y, yh, st))
